# revision 16
# baseline (speedup 1.0000x reference)
"""ASPP-Deformable Trainium2 kernel (nn_ASPPDeformable_52046413693303).

Strategy
--------
8 NeuronCores = 2 batches x 4 pixel-quarters (1024 pixels each).

Deformable sampling uses the 4-plane identity: with zero-extended image I~,
Dx = I~(y,x+1)-I~(y,x), Dy = I~(y+1,x)-I~(y,x), Dxy = Dx(y+1,x)-Dx(y,x):

    bilinear(py, px) = I~(y0,x0) + wx*Dx(y0,x0) + wy*Dy(y0,x0) + wx*wy*Dxy(y0,x0)

so one gathered row of a precomputed 4-plane table [I|Dx|Dy|Dxy] (4*C bf16 =
1KB) yields a full bilinear sample.  The table lives on an 80x80 grid
(margin 8 on each side of the 64x64 image); clamping y0/x0 into the margin
reproduces the reference's zero-padding exactly (all margin rows are zero).

Pipeline per core:
  - build 4-plane table in C-major SBUF, xbar-transpose, write to DRAM
  - offset/modulator convs as im2col matmuls (pixel-major out)
  - per-pixel index+lerp-coef pipeline on DVE/ACT
  - int16 indices rewrapped to dma_gather's 16-partition layout via strided
    SBUF->SBUF DMAs, replicated 8x
  - dma_gather (1KB rows), fused lerp via scalar_tensor_tensor with
    per-partition (= per-pixel) coefficients, xbar transpose to C-major,
    bf16 matmul with regw accumulated over kernel points in PSUM
  - GroupNorm stats partial sums AllReduce'd across the 4 cores of a batch
  - concat (C-major) -> fuse 1x1 conv -> fuse GN (2nd AllReduce) -> ReLU
"""
import os
import sys
import numpy as np
import ml_dtypes

sys.path.insert(0, "/opt/trn_rl_repo")

import concourse.bacc as bacc  # noqa: E402
import concourse.bass as bass  # noqa: E402
import concourse.tile as tile  # noqa: E402
from concourse import mybir  # noqa: E402
from concourse.bass_utils import run_bass_kernel_spmd  # noqa: E402

f32 = mybir.dt.float32
bf16 = mybir.dt.bfloat16
i16 = mybir.dt.int16
i32 = mybir.dt.int32
ALU = mybir.AluOpType
ACTF = mybir.ActivationFunctionType

B, C, H, W = 2, 128, 64, 64
HW = H * W
O = 256
NGRP = 16
EPS = 1e-5
MARGIN = 8
GRID = H + 2 * MARGIN          # 80
GRID2 = GRID * GRID            # 6400
QROWS = H // 4                 # 16 image rows per core
QPIX = QROWS * W               # 1024 pixels per core
NT = QPIX // 128               # 8 pixel tiles of 128 per core
E4 = 4 * C                     # table row length (elements)

# branch configs: (name, k, pad)
BRANCHES = [("b1", 1, 0), ("b2", 1, 0), ("b3", 3, 1), ("b4", 7, 3)]
HALO = 3                       # max pad over branches
HROWS = QROWS + 2 * HALO       # 22 halo rows
HCOLS = W + 2 * HALO           # 70 halo cols

_CACHE = {}


JGMAX = 13


def _jgroups(K):
    """Split kernel points into gather groups (<=JGMAX j per group)."""
    ngr = -(-K // JGMAX)
    base = K // ngr
    rem = K % ngr
    out, j = [], 0
    for g in range(ngr):
        n = base + (1 if g < rem else 0)
        out.append(list(range(j, j + n)))
        j += n
    return out


def build_program():
    nc = bacc.Bacc("TRN2", target_bir_lowering=False, num_devices=8)

    # ---------------- I/O declarations (per-core data supplied by host) ----
    inp = {}

    def di(name, shape, dtype):
        inp[name] = nc.dram_tensor(name, shape, dtype, kind="ExternalInput")
        return inp[name]

    di("pad80", [C, GRID2], bf16)           # zero-padded bf16 image, C-major
    di("x_halo", [C, HROWS * HCOLS], bf16)  # quarter + halo, zero padded
    for name, k, pad in BRANCHES:
        K = k * k
        CH = 3 * K
        di(f"{name}_wstk", [C, K, CH], bf16)      # [c, tap, out_ch]
        di(f"{name}_bias", [1, CH], bf16)         # offb|modb
        di(f"{name}_regw", [C, K, O], bf16)       # 2*regw, [c, j, o]
        di(f"{name}_by16", [128, NT, K], f32)     # base_y + 16 per (pix, t, j)
        di(f"{name}_bx16", [128, NT, K], f32)
        di(f"{name}_gamma", [1, O], f32)
        di(f"{name}_beta", [1, O], f32)
    di("pool_w", [C, O], f32)
    di("pool_gamma", [1, O], f32)
    di("pool_beta", [1, O], f32)
    di("fuse_wT", [C, 10, 128], bf16)        # chunk i: fuse_w[:, i*128+ch].T
    di("fuse_gamma", [1, 128], f32)
    di("fuse_beta", [1, 128], f32)

    out_d = nc.dram_tensor("out", [QPIX, 128], f32, kind="ExternalOutput")

    Ttbl = nc.dram_tensor("Ttbl", [GRID2, E4], bf16)

    with tile.TileContext(nc) as tc:
        with (
            tc.tile_pool(name="consts", bufs=1) as cst,
            tc.tile_pool(name="bigio", bufs=1) as bigio,
            tc.tile_pool(name="plane", bufs=1) as plane_p,
            tc.tile_pool(name="wts", bufs=1) as wts,
            tc.tile_pool(name="offs", bufs=1) as offs_p,
            tc.tile_pool(name="coef", bufs=1) as coef_p,
            tc.tile_pool(name="wrap", bufs=1) as wrap_p,
            tc.tile_pool(name="gat", bufs=2) as gat_p,
            tc.tile_pool(name="lrp", bufs=2) as lrp_p,
            tc.tile_pool(name="keep", bufs=1) as keep,
            tc.tile_pool(name="tmp", bufs=1) as tmp,
            tc.tile_pool(name="psA", bufs=2, space="PSUM") as psA,
            tc.tile_pool(name="psB", bufs=2, space="PSUM") as psB,
            tc.tile_pool(name="psS", bufs=1, space="PSUM") as psS,
            tc.tile_pool(name="dram", bufs=1, space="DRAM") as drm,
        ):
            # ---------------- constants -----------------------------------
            ones_col = cst.tile([128, 1], bf16, tag="ones_col")
            nc.vector.memset(ones_col[:], 1.0)
            ones_col_f = cst.tile([128, 1], f32, tag="ones_col_f")
            nc.vector.memset(ones_col_f[:], 1.0)
            ones_row = cst.tile([1, 128], bf16, tag="ones_row")
            nc.vector.memset(ones_row[:], 1.0)

            # ---------------- pool branch --------------------------------
            xsum = keep.tile([128, 1], f32, tag="xsum")
            xmean = keep.tile([128, 1], f32, tag="xmean")
            # ---------------- 4-plane table build -------------------------
            p80 = bigio.tile([C, GRID2], bf16, tag="pad80")
            nc.sync.dma_start(p80[:], inp["pad80"][:])
            nc.vector.tensor_reduce(
                out=xsum[:],
                in_=p80[:].rearrange("c (gy gx) -> c gy gx", gx=GRID)[
                    :, MARGIN : MARGIN + H, MARGIN : MARGIN + W
                ],
                axis=mybir.AxisListType.XY,
                op=ALU.add,
            )
            nc.vector.tensor_scalar_mul(xmean[:], xsum[:], 1.0 / HW)
            planes = []
            pI = p80
            dx = gat_p.tile([C, GRID2], bf16, tag="gath")
            nc.vector.memset(dx[:, GRID2 - 1 :], 0.0)
            nc.vector.tensor_tensor(
                out=dx[:, : GRID2 - 1],
                in0=p80[:, 1:GRID2],
                in1=p80[:, : GRID2 - 1],
                op=ALU.subtract,
            )
            dy = gat_p.tile([C, GRID2], bf16, tag="gath")
            nc.gpsimd.memset(dy[:, GRID2 - GRID :], 0.0)
            nc.vector.tensor_tensor(
                out=dy[:, : GRID2 - GRID],
                in0=p80[:, GRID:GRID2],
                in1=p80[:, : GRID2 - GRID],
                op=ALU.subtract,
            )
            dxy = gat_p.tile([C, GRID2], bf16, tag="gath")
            nc.gpsimd.memset(dxy[:, GRID2 - GRID :], 0.0)
            nc.vector.tensor_tensor(
                out=dxy[:, : GRID2 - GRID],
                in0=dx[:, GRID:GRID2],
                in1=dx[:, : GRID2 - GRID],
                op=ALU.subtract,
            )
            planes = [pI, dx, dy, dxy]
            poolw_sb = wts.tile([C, O], f32, tag="poolw")
            nc.sync.dma_start(poolw_sb[:], inp["pool_w"][:])
            pp_ps = psS.tile([1, O], f32, tag="rowp")
            nc.tensor.matmul(pp_ps[:], xmean[:], poolw_sb[:], start=True, stop=True)
            pp = keep.tile([1, O], f32, tag="poolrow")
            nc.vector.tensor_copy(pp[:], pp_ps[:])
            # GN over each group of 16 channels (spatial = 1)
            ps_sum = keep.tile([1, NGRP], f32, tag="pool_s")
            nc.vector.tensor_reduce(
                out=ps_sum[:],
                in_=pp[:].rearrange("a (g d) -> a g d", g=NGRP),
                axis=mybir.AxisListType.X,
                op=ALU.add,
            )
            pp_sq = keep.tile([1, O], f32, tag="gn_msb")
            nc.scalar.activation(out=pp_sq[:], in_=pp[:], func=ACTF.Square)
            ps_ssq = keep.tile([1, NGRP], f32, tag="pool_ss")
            nc.vector.tensor_reduce(
                out=ps_ssq[:],
                in_=pp_sq[:].rearrange("a (g d) -> a g d", g=NGRP),
                axis=mybir.AxisListType.X,
                op=ALU.add,
            )

            def gn_scale_bias(s_ap, ss_ap, count, gamma_ap, beta_ap, nch, ngrp, tagp):
                """Given group sums/sumsqs [1, ngrp], returns per-channel
                scale/bias rows [1, nch] (f32, partition 0)."""
                g = ngrp
                mean = keep.tile([1, NGRP], f32, tag="gn_mean")
                nc.vector.tensor_scalar_mul(mean[:, :g], s_ap, 1.0 / count)
                ex2 = keep.tile([1, NGRP], f32, tag="gn_ex2")
                nc.vector.tensor_scalar_mul(ex2[:, :g], ss_ap, 1.0 / count)
                msq = keep.tile([1, NGRP], f32, tag="gn_msq")
                nc.scalar.activation(out=msq[:, :g], in_=mean[:, :g], func=ACTF.Square)
                var = keep.tile([1, NGRP], f32, tag="gn_var")
                nc.vector.tensor_tensor(
                    out=var[:, :g], in0=ex2[:, :g], in1=msq[:, :g], op=ALU.subtract
                )
                nc.vector.tensor_scalar_add(var[:, :g], var[:, :g], float(EPS))
                std = keep.tile([1, NGRP], f32, tag="gn_std")
                nc.scalar.activation(out=std[:, :g], in_=var[:, :g], func=ACTF.Sqrt)
                rstd = keep.tile([1, NGRP], f32, tag="gn_rstd")
                nc.vector.reciprocal(rstd[:, :g], std[:, :g])
                d = nch // g
                scale_t = keep.tile([1, O], f32, tag="gn_scale")
                scale = scale_t[:, :nch]
                rstd_b = rstd[:, :g].rearrange("a (g o) -> a g o", o=1).to_broadcast([1, g, d])
                nc.vector.tensor_tensor(
                    out=scale.rearrange("a (g d) -> a g d", g=g),
                    in0=rstd_b,
                    in1=gamma_ap.rearrange("a (g d) -> a g d", g=g),
                    op=ALU.mult,
                )
                mean_b = mean[:, :g].rearrange("a (g o) -> a g o", o=1).to_broadcast([1, g, d])
                msb_t = keep.tile([1, O], f32, tag="gn_msb")
                msb = msb_t[:, :nch]
                nc.vector.tensor_tensor(
                    out=msb.rearrange("a (g d) -> a g d", g=g),
                    in0=mean_b,
                    in1=scale.rearrange("a (g d) -> a g d", g=g),
                    op=ALU.mult,
                )
                bias_t = keep.tile([1, O], f32, tag="gn_bias")
                bias = bias_t[:, :nch]
                nc.vector.tensor_tensor(
                    out=bias, in0=beta_ap, in1=msb, op=ALU.subtract
                )
                return scale, bias

            pgam = keep.tile([1, O], f32, tag="pool_gam")
            nc.sync.dma_start(pgam[:], inp["pool_gamma"][:])
            pbet = keep.tile([1, O], f32, tag="pool_bet")
            nc.sync.dma_start(pbet[:], inp["pool_beta"][:])
            psc, pbi = gn_scale_bias(
                ps_sum[:], ps_ssq[:], 16.0, pgam[:], pbet[:], O, NGRP, "pool"
            )
            x5row = keep.tile([1, O], bf16, tag="x5row")
            x5t = keep.tile([1, O], f32, tag="x5t")
            nc.vector.tensor_tensor(out=x5t[:], in0=pp[:], in1=psc, op=ALU.mult)
            nc.vector.tensor_tensor(out=x5t[:], in0=x5t[:], in1=pbi, op=ALU.add)
            nc.scalar.activation(out=x5row[:], in_=x5t[:], func=ACTF.Relu)

            # cat buffer: [128 pix, t, 10 chunks, 128 ch] bf16
            cat = keep.tile([128, NT, 8, 128], bf16, tag="cat")
            x5cm2 = keep.tile([128, 2, 128], bf16, tag="x5cm2")
            # x5 C-major chunks via rank-1 matmul (x5col x ones_row)
            for hchunk in range(2):
                x5ps = psS.tile([128, 128], f32, tag="x5ps")
                nc.tensor.matmul(
                    x5ps[:],
                    x5row[0:1, 128 * hchunk : 128 * (hchunk + 1)],
                    ones_row[:],
                    start=True,
                    stop=True,
                )
                nc.vector.tensor_copy(x5cm2[:, hchunk, :], x5ps[:])
            UBLK = GRID2 // 128  # 50
            Tv = Ttbl.ap().rearrange("(u a) e -> a u e", a=128)
            for p_i, pl in enumerate(planes):
                pt = plane_p.tile([128, UBLK, 128], bf16, tag="plane_t")
                nc.sync.dma_start_transpose(pt[:], pl[:])
                nc.sync.dma_start(Tv[:, :, 128 * p_i : 128 * (p_i + 1)], pt[:])

            # ---------------- branches ------------------------------------
            xh = bigio.tile([C, HROWS, HCOLS], bf16, tag="xhalo")
            nc.sync.dma_start(
                xh[:].rearrange("c r d -> c (r d)"), inp["x_halo"][:]
            )

            # collective staging
            cc1_in = drm.tile([1, 4 * 2 * NGRP], f32)
            cc1_out = drm.tile([1, 4 * 2 * NGRP], f32)
            cc2_in = drm.tile([1, 2 * NGRP], f32)
            cc2_out = drm.tile([1, 2 * NGRP], f32)

            out_br = []  # per-branch [128, NT, O] f32 (pre-GN)
            for bi, (name, k, pad) in enumerate(BRANCHES):
                K = k * k
                CH = 3 * K
                # ---- weights
                wstk = wts.tile([C, K, CH], bf16, tag="wstk")
                nc.sync.dma_start(wstk[:], inp[f"{name}_wstk"][:])
                brow = wts.tile([1, CH], bf16, tag="brow")
                nc.sync.dma_start(brow[:], inp[f"{name}_bias"][:])
                regw = wts.tile([C, K, O], bf16, tag="regw")
                nc.sync.dma_start(regw[:], inp[f"{name}_regw"][:])
                by16 = coef_p.tile([128, NT, K], f32, tag="by16")
                nc.sync.dma_start(by16[:], inp[f"{name}_by16"][:])
                bx16 = coef_p.tile([128, NT, K], f32, tag="bx16")
                nc.sync.dma_start(bx16[:], inp[f"{name}_bx16"][:])

                # ---- offset/mod convs: out^T [pix, CH] per tile
                offs = offs_p.tile([128, NT, CH], f32, tag="offs")
                for t in range(NT):
                    op_ps = psA.tile([128, 256], f32, tag="convps")
                    first = True
                    for ky in range(k):
                        for kx in range(k):
                            tap = ky * k + kx
                            r0 = 2 * t + ky - pad + HALO
                            c0 = kx - pad + HALO
                            for rr in range(2):
                                nc.tensor.matmul(
                                    op_ps[64 * rr : 64 * (rr + 1), :CH],
                                    xh[:, r0 + rr, c0 : c0 + W],
                                    wstk[:, tap, :],
                                    start=first,
                                    stop=False,
                                )
                            first = False
                    nc.tensor.matmul(
                        op_ps[:, :CH], ones_row[:], brow[:], start=False, stop=True
                    )
                    nc.vector.tensor_copy(offs[:, t, :], op_ps[:, :CH])

                # ---- mask + coefs + indices, all tiles at once [128, NT*K]
                mask = coef_p.tile([128, NT, K], f32, tag="mask")
                nc.scalar.activation(
                    out=mask[:], in_=offs[:, :, 2 * K : 3 * K], func=ACTF.Sigmoid
                )
                py = coef_p.tile([128, NT, K], f32, tag="py")
                nc.vector.tensor_tensor(
                    out=py[:], in0=offs[:, :, 0 : 2 * K : 2], in1=by16[:], op=ALU.add
                )
                px = coef_p.tile([128, NT, K], f32, tag="px")
                nc.vector.tensor_tensor(
                    out=px[:], in0=offs[:, :, 1 : 2 * K : 2], in1=bx16[:], op=ALU.add
                )
                fy = coef_p.tile([128, NT, K], f32, tag="fy")
                fx = coef_p.tile([128, NT, K], f32, tag="fx")
                wy = coef_p.tile([128, NT, K], f32, tag="wy")
                wx = coef_p.tile([128, NT, K], f32, tag="wx")
                for crd, (pt_, ft_, wt_) in {
                    "y": (py, fy, wy),
                    "x": (px, fx, wx),
                }.items():
                    # clamp into the table grid
                    nc.vector.tensor_scalar_max(pt_[:], pt_[:], float(MARGIN))
                    nc.vector.tensor_scalar_min(
                        pt_[:], pt_[:], float(H + MARGIN - 2 + 16) + 0.99
                    )
                    # floor via int cast + correction (works for trunc/round)
                    icast = coef_p.tile([128, NT, K], i32, tag="scr")
                    corr = coef_p.tile([128, NT, K], f32, tag="scr")
                    nc.vector.tensor_copy(icast[:], pt_[:])
                    nc.vector.tensor_copy(ft_[:], icast[:])
                    nc.vector.tensor_tensor(
                        out=corr[:], in0=ft_[:], in1=pt_[:], op=ALU.is_gt
                    )
                    nc.vector.tensor_tensor(
                        out=ft_[:], in0=ft_[:], in1=corr[:], op=ALU.subtract
                    )
                    nc.vector.tensor_tensor(
                        out=wt_[:], in0=pt_[:], in1=ft_[:], op=ALU.subtract
                    )
                # idx = GRID*fy + fx - (16*GRID + 16 - 8*GRID - 8)
                idxf = coef_p.tile([128, NT, K], f32, tag="scr")
                nc.vector.scalar_tensor_tensor(
                    out=idxf[:],
                    in0=fy[:],
                    scalar=float(GRID),
                    in1=fx[:],
                    op0=ALU.mult,
                    op1=ALU.add,
                )
                nc.vector.tensor_scalar_add(
                    idxf[:], idxf[:], -float((16 - MARGIN) * GRID + (16 - MARGIN))
                )
                idx16 = coef_p.tile([128, NT, K], i16, tag="idx16")
                nc.vector.tensor_copy(idx16[:], idxf[:])
                # lerp coefs (x2 sigmoid scale folded into regw)
                c1 = coef_p.tile([128, NT, K], f32, tag="c1")
                nc.vector.tensor_tensor(out=c1[:], in0=mask[:], in1=wx[:], op=ALU.mult)
                c2 = coef_p.tile([128, NT, K], f32, tag="c2")
                nc.vector.tensor_tensor(out=c2[:], in0=mask[:], in1=wy[:], op=ALU.mult)
                c3 = coef_p.tile([128, NT, K], f32, tag="c3")
                nc.vector.tensor_tensor(out=c3[:], in0=c1[:], in1=wy[:], op=ALU.mult)

                # ---- index wrap: wrapcols[(t,j)] block of 8, order (t, j, r)
                WCOLS = NT * K * 8
                wrapt = wrap_p.tile([128, WCOLS], i16, tag="wrapt")
                wv = wrapt[:].rearrange("p (t j r) -> p t j r", t=NT, j=K)
                for r in range(8):
                    nc.sync.dma_start(
                        wv[0:16, :, :, r], idx16[16 * r : 16 * r + 16, :, :]
                    )
                for a in range(1, 8):
                    nc.sync.dma_start(
                        wrapt[16 * a : 16 * a + 16, :], wrapt[0:16, :]
                    )

                # ---- gather / lerp / transpose / matmul ------------------
                jgs = _jgroups(K)
                sums = keep.tile([128, 2 * NGRP], f32, tag="stats")
                nc.vector.memset(sums[:], 0.0)
                obr = keep.tile([128, NT, O], bf16, tag=f"outbr{bi}")
                out_br.append(obr)
                for t in range(NT):
                    ps_out = psB.tile([128, O], f32, tag="ps_out")
                    for gi, jg in enumerate(jgs):
                        j0, nj = jg[0], len(jg)
                        gt = gat_p.tile([128, JGMAX, E4], bf16, tag="gath")
                        nidx = nj * 128
                        nc.gpsimd.dma_gather(
                            out_ap=gt[:, :nj, :],
                            in_ap=Ttbl.ap(),
                            idxs_ap=wrapt[:, (t * K + j0) * 8 : (t * K + j0 + nj) * 8],
                            num_idxs=nidx,
                            num_idxs_reg=nidx,
                            elem_size=E4,
                            single_packet=False,
                        )
                        st = lrp_p.tile([128, JGMAX, 128], bf16, tag="stile")
                        acc = lrp_p.tile([128, JGMAX, 128], f32, tag="sacc")
                        for jj in range(nj):
                            j = j0 + jj
                            msc = mask[:, t, j : j + 1]
                            c1s = c1[:, t, j : j + 1]
                            c2s = c2[:, t, j : j + 1]
                            c3s = c3[:, t, j : j + 1]
                            gI = gt[:, jj, 0:128]
                            gDx = gt[:, jj, 128:256]
                            gDy = gt[:, jj, 256:384]
                            gDxy = gt[:, jj, 384:512]
                            a_ = acc[:, jj, :]
                            if jj % 2 == 0:
                                nc.scalar.activation(
                                    out=a_, in_=gI, func=ACTF.Copy, scale=msc
                                )
                            else:
                                nc.vector.tensor_scalar_mul(a_, gI, msc)
                            nc.vector.scalar_tensor_tensor(
                                out=a_, in0=gDx, scalar=c1s, in1=a_,
                                op0=ALU.mult, op1=ALU.add,
                            )
                            nc.vector.scalar_tensor_tensor(
                                out=a_, in0=gDy, scalar=c2s, in1=a_,
                                op0=ALU.mult, op1=ALU.add,
                            )
                            nc.vector.scalar_tensor_tensor(
                                out=st[:, jj, :], in0=gDxy, scalar=c3s, in1=a_,
                                op0=ALU.mult, op1=ALU.add,
                            )
                        stT = lrp_p.tile([128, JGMAX, 128], bf16, tag="stileT")
                        nc.sync.dma_start_transpose(
                            stT[:, :nj, :],
                            st[:, :nj, :].rearrange("p u e -> p (u e)"),
                        )
                        for jj in range(nj):
                            j = j0 + jj
                            nc.tensor.matmul(
                                ps_out[:],
                                stT[:, jj, :],
                                regw[:, j, :],
                                start=(j == 0),
                                stop=(j == K - 1),
                            )
                    # ---- epilogue for tile t: stats + stash + GN later
                    nc.vector.tensor_copy(obr[:, t, :], ps_out[:])
                    sq = tmp.tile([128, O], f32, tag="sqtile")
                    nc.scalar.activation(out=sq[:], in_=ps_out[:], func=ACTF.Square)
                    part = tmp.tile([128, 2 * NGRP], f32, tag="part")
                    nc.vector.tensor_reduce(
                        out=part[:, :NGRP],
                        in_=ps_out[:].rearrange("p (g d) -> p g d", g=NGRP),
                        axis=mybir.AxisListType.X,
                        op=ALU.add,
                    )
                    nc.vector.tensor_reduce(
                        out=part[:, NGRP:],
                        in_=sq[:].rearrange("p (g d) -> p g d", g=NGRP),
                        axis=mybir.AxisListType.X,
                        op=ALU.add,
                    )
                    nc.vector.tensor_tensor(
                        out=sums[:], in0=sums[:], in1=part[:], op=ALU.add
                    )
                # partition-reduce stats -> [2*NGRP, 1] -> DRAM staging
                sred = psS.tile([2 * NGRP, 1], f32, tag="sred")
                nc.tensor.matmul(sred[:], sums[:], ones_col_f[:], start=True, stop=True)
                sred_sb = tmp.tile([2 * NGRP, 1], f32, tag="sred_sb")
                nc.vector.tensor_copy(sred_sb[:], sred[:])
                nc.sync.dma_start(
                    cc1_in[0:1, bi * 2 * NGRP : (bi + 1) * 2 * NGRP].rearrange(
                        "a e -> e a"
                    ),
                    sred_sb[:],
                )

            # ---------------- AllReduce #1 --------------------------------
            nc.gpsimd.collective_compute(
                "AllReduce",
                ALU.add,
                replica_groups=[[0, 1, 2, 3], [4, 5, 6, 7]],
                ins=[cc1_in[:]],
                outs=[cc1_out[:]],
            )
            cc1_sb = keep.tile([1, 4 * 2 * NGRP], f32, tag="cc1_sb")
            nc.sync.dma_start(cc1_sb[:], cc1_out[:])

            # ---------------- GN apply per branch + cat -------------------
            for bi, (name, k, pad) in enumerate(BRANCHES):
                gam = keep.tile([1, O], f32, tag="gamld")
                nc.sync.dma_start(gam[:], inp[f"{name}_gamma"][:])
                bet = keep.tile([1, O], f32, tag="betld")
                nc.sync.dma_start(bet[:], inp[f"{name}_beta"][:])
                s_ap = cc1_sb[0:1, bi * 2 * NGRP : bi * 2 * NGRP + NGRP]
                ss_ap = cc1_sb[0:1, bi * 2 * NGRP + NGRP : (bi + 1) * 2 * NGRP]
                scale, bias = gn_scale_bias(
                    s_ap, ss_ap, float(16 * HW), gam[:], bet[:], O, NGRP, "br"
                )
                scb = keep.tile([128, O], f32, tag="scb")
                nc.gpsimd.partition_broadcast(scb[:], scale)
                bib = keep.tile([128, O], f32, tag="bib")
                nc.gpsimd.partition_broadcast(bib[:], bias)
                obr = out_br[bi]
                for t in range(NT):
                    nrm = tmp.tile([128, O], f32, tag="nrm")
                    nc.vector.tensor_tensor(
                        out=nrm[:], in0=obr[:, t, :], in1=scb[:], op=ALU.mult
                    )
                    nc.vector.tensor_tensor(
                        out=nrm[:], in0=nrm[:], in1=bib[:], op=ALU.add
                    )
                    nrmb = tmp.tile([128, O], bf16, tag="nrmb")
                    nc.scalar.activation(out=nrmb[:], in_=nrm[:], func=ACTF.Relu)
                    nc.sync.dma_start_transpose(
                        cat[:, t, 2 * bi : 2 * bi + 2, :], nrmb[:]
                    )

            # ---------------- fuse conv + GN ------------------------------
            fw = wts.tile([C, 10, 128], bf16, tag="fusew")
            nc.sync.dma_start(fw[:], inp["fuse_wT"][:])
            fsums = keep.tile([128, 2 * NGRP], f32, tag="fsums")
            nc.vector.memset(fsums[:], 0.0)
            fout = keep.tile([128, NT, 128], bf16, tag="fout")
            for t in range(NT):
                fps = psB.tile([128, O], f32, tag="ps_out")
                for i in range(10):
                    lhsT = cat[:, t, i, :] if i < 8 else x5cm2[:, i - 8, :]
                    nc.tensor.matmul(
                        fps[:, :128],
                        lhsT,
                        fw[:, i, :],
                        start=(i == 0),
                        stop=(i == 9),
                    )
                nc.vector.tensor_copy(fout[:, t, :], fps[:, :128])
                fsq = tmp.tile([128, 128], f32, tag="fsq")
                nc.scalar.activation(out=fsq[:], in_=fps[:, :128], func=ACTF.Square)
                fpart = tmp.tile([128, 2 * NGRP], f32, tag="fpart")
                nc.vector.tensor_reduce(
                    out=fpart[:, :NGRP],
                    in_=fps[:, :128].rearrange("p (g d) -> p g d", g=NGRP),
                    axis=mybir.AxisListType.X,
                    op=ALU.add,
                )
                nc.vector.tensor_reduce(
                    out=fpart[:, NGRP:],
                    in_=fsq[:].rearrange("p (g d) -> p g d", g=NGRP),
                    axis=mybir.AxisListType.X,
                    op=ALU.add,
                )
                nc.vector.tensor_tensor(
                    out=fsums[:], in0=fsums[:], in1=fpart[:], op=ALU.add
                )
            fred = psS.tile([2 * NGRP, 1], f32, tag="sred")
            nc.tensor.matmul(fred[:], fsums[:], ones_col_f[:], start=True, stop=True)
            fred_sb = tmp.tile([2 * NGRP, 1], f32, tag="sred_sb")
            nc.vector.tensor_copy(fred_sb[:], fred[:])
            nc.sync.dma_start(cc2_in[:].rearrange("a e -> e a"), fred_sb[:])
            nc.gpsimd.collective_compute(
                "AllReduce",
                ALU.add,
                replica_groups=[[0, 1, 2, 3], [4, 5, 6, 7]],
                ins=[cc2_in[:]],
                outs=[cc2_out[:]],
            )
            cc2_sb = keep.tile([1, 2 * NGRP], f32, tag="cc2_sb")
            nc.sync.dma_start(cc2_sb[:], cc2_out[:])
            fgam_t = keep.tile([1, O], f32, tag="gamld")
            fgam = fgam_t[:, :128]
            nc.sync.dma_start(fgam, inp["fuse_gamma"][:])
            fbet_t = keep.tile([1, O], f32, tag="betld")
            fbet = fbet_t[:, :128]
            nc.sync.dma_start(fbet, inp["fuse_beta"][:])
            fscale, fbias = gn_scale_bias(
                cc2_sb[0:1, :NGRP],
                cc2_sb[0:1, NGRP:],
                float(8 * HW),
                fgam,
                fbet,
                128,
                NGRP,
                "fuse",
            )
            fscb_t = keep.tile([128, O], f32, tag="scb")
            fscb = fscb_t[:, :128]
            nc.gpsimd.partition_broadcast(fscb, fscale)
            fbib_t = keep.tile([128, O], f32, tag="bib")
            fbib = fbib_t[:, :128]
            nc.gpsimd.partition_broadcast(fbib, fbias)
            for t in range(NT):
                fn = tmp.tile([128, 128], f32, tag="fn")
                nc.vector.tensor_tensor(
                    out=fn[:], in0=fout[:, t, :], in1=fscb, op=ALU.mult
                )
                nc.vector.tensor_tensor(out=fn[:], in0=fn[:], in1=fbib, op=ALU.add)
                fr = tmp.tile([128, 128], f32, tag="fr")
                nc.scalar.activation(out=fr[:], in_=fn[:], func=ACTF.Relu)
                nc.sync.dma_start(out_d[128 * t : 128 * (t + 1), :], fr[:])

    nc.compile()
    return nc


# ======================= host-side data prep ==============================

def _prep_core_inputs(inputs, core):
    b, q = core // 4, core % 4
    x = np.asarray(inputs["x"])[b]            # [C, H, W] f32
    xc = x.reshape(C, HW).astype(np.float32)
    m = {}
    pad = np.zeros((C, GRID, GRID), np.float32)
    pad[:, MARGIN : MARGIN + H, MARGIN : MARGIN + W] = x
    m["pad80"] = pad.reshape(C, GRID2).astype(ml_dtypes.bfloat16)
    halo = np.zeros((C, HROWS, HCOLS), np.float32)
    r0 = q * QROWS - HALO
    for rr in range(HROWS):
        rimg = r0 + rr
        if 0 <= rimg < H:
            halo[:, rr, HALO : HALO + W] = x[:, rimg, :]
    m["x_halo"] = halo.reshape(C, HROWS * HCOLS).astype(ml_dtypes.bfloat16)

    yq = np.arange(QPIX) // W + q * QROWS     # image row per pixel
    xq = np.arange(QPIX) % W
    # pixel i at (partition i%128, tile i//128)
    yq = yq.reshape(NT, 128).T                # [128, NT]
    xq = xq.reshape(NT, 128).T

    for name, k, pad_ in BRANCHES:
        K = k * k
        offw = np.asarray(inputs[f"{name}_offw"], np.float32)
        offb = np.asarray(inputs[f"{name}_offb"], np.float32)
        modw = np.asarray(inputs[f"{name}_modw"], np.float32)
        modb = np.asarray(inputs[f"{name}_modb"], np.float32)
        regw = np.asarray(inputs[f"{name}_regw"], np.float32)
        CH = 3 * K
        wstk = np.zeros((C, K, CH), np.float32)
        for ky in range(k):
            for kx in range(k):
                tap = ky * k + kx
                wstk[:, tap, : 2 * K] = offw[:, :, ky, kx].T
                wstk[:, tap, 2 * K :] = modw[:, :, ky, kx].T
        m[f"{name}_wstk"] = wstk.astype(ml_dtypes.bfloat16)
        m[f"{name}_bias"] = np.concatenate([offb, modb])[None, :].astype(
            ml_dtypes.bfloat16
        )
        m[f"{name}_regw"] = (
            (2.0 * regw.reshape(O, C, K)).transpose(1, 2, 0).astype(ml_dtypes.bfloat16)
        )
        ky, kx = np.meshgrid(np.arange(k), np.arange(k), indexing="ij")
        ky = ky.reshape(-1).astype(np.float32)
        kx = kx.reshape(-1).astype(np.float32)
        m[f"{name}_by16"] = (
            yq[:, :, None] - pad_ + ky[None, None, :] + 16.0
        ).astype(np.float32)
        m[f"{name}_bx16"] = (
            xq[:, :, None] - pad_ + kx[None, None, :] + 16.0
        ).astype(np.float32)
        m[f"{name}_gamma"] = np.asarray(inputs[f"{name}_gng"], np.float32)[None, :]
        m[f"{name}_beta"] = np.asarray(inputs[f"{name}_gnb"], np.float32)[None, :]

    m["pool_w"] = np.asarray(inputs["pool_w"], np.float32).reshape(O, C).T.copy()
    m["pool_gamma"] = np.asarray(inputs["pool_gng"], np.float32)[None, :]
    m["pool_beta"] = np.asarray(inputs["pool_gnb"], np.float32)[None, :]
    fw = np.asarray(inputs["fuse_w"], np.float32).reshape(128, 1280)
    fwT = np.zeros((C, 10, 128), np.float32)
    for i in range(10):
        fwT[:, i, :] = fw[:, i * 128 : (i + 1) * 128].T
    m["fuse_wT"] = fwT.astype(ml_dtypes.bfloat16)
    m["fuse_gamma"] = np.asarray(inputs["fuse_gng"], np.float32)[None, :]
    m["fuse_beta"] = np.asarray(inputs["fuse_gnb"], np.float32)[None, :]
    return m


def kernel(**inputs):
    if "nc" not in _CACHE:
        _CACHE["nc"] = build_program()
    nc = _CACHE["nc"]
    in_maps = [_prep_core_inputs(inputs, core) for core in range(8)]
    res = run_bass_kernel_spmd(nc, in_maps, list(range(8)))
    _CACHE["last_results"] = res
    out = np.zeros((B, 128, H, W), np.float32)
    for core in range(8):
        b, q = core // 4, core % 4
        oc = res.results[core]["out"]          # [QPIX, 128]
        # pixel i at (partition i%128, row-tile i//128): out rows are pixel-major
        oc = oc.reshape(QROWS, W, 128).transpose(2, 0, 1)
        out[b, :, q * QROWS : (q + 1) * QROWS, :] = oc
    return out


# revision 45
# speedup vs baseline: 3206.5966x; 3206.5966x over previous
"""ASPP-Deformable Trainium2 kernel (nn_ASPPDeformable_52046413693303).

Strategy
--------
8 NeuronCores = 2 batches x 4 pixel-quarters (1024 pixels each).

Deformable sampling uses the 4-plane identity: with zero-extended image I~,
Dx = I~(y,x+1)-I~(y,x), Dy = I~(y+1,x)-I~(y,x), Dxy = Dx(y+1,x)-Dx(y,x):

    bilinear(py, px) = I~(y0,x0) + wx*Dx(y0,x0) + wy*Dy(y0,x0) + wx*wy*Dxy(y0,x0)

so one gathered row of a precomputed 4-plane table [I|Dx|Dy|Dxy] (4*C bf16 =
1KB) yields a full bilinear sample.  The table lives on an 80x80 grid
(margin 8 on each side of the 64x64 image); clamping y0/x0 into the margin
reproduces the reference's zero-padding exactly (all margin rows are zero).

Pipeline per core:
  - build 4-plane table in C-major SBUF, xbar-transpose, write to DRAM
  - offset/modulator convs as im2col matmuls (pixel-major out)
  - per-pixel index+lerp-coef pipeline on DVE/ACT
  - int16 indices rewrapped to dma_gather's 16-partition layout via strided
    SBUF->SBUF DMAs, replicated 8x
  - dma_gather (1KB rows), fused lerp via scalar_tensor_tensor with
    per-partition (= per-pixel) coefficients, xbar transpose to C-major,
    bf16 matmul with regw accumulated over kernel points in PSUM
  - GroupNorm stats partial sums AllReduce'd across the 4 cores of a batch
  - concat (C-major) -> fuse 1x1 conv -> fuse GN (2nd AllReduce) -> ReLU
"""
import os
import sys
import numpy as np
import ml_dtypes

sys.path.insert(0, "/opt/trn_rl_repo")

import concourse.bacc as bacc  # noqa: E402
import concourse.bass as bass  # noqa: E402
import concourse.tile as tile  # noqa: E402
from concourse import mybir  # noqa: E402
from concourse.bass_utils import run_bass_kernel_spmd  # noqa: E402

f32 = mybir.dt.float32
bf16 = mybir.dt.bfloat16
i16 = mybir.dt.int16
i32 = mybir.dt.int32
ALU = mybir.AluOpType
ACTF = mybir.ActivationFunctionType

B, C, H, W = 2, 128, 64, 64
HW = H * W
O = 256
NGRP = 16
EPS = 1e-5
MARGIN = 8
GRID = H + 2 * MARGIN          # 80 (x direction, full width)
BROWS = 32                     # band rows per core (y0_local in [-8, 24))
BSRC = BROWS + 1               # +1 row so Dy/Dxy shifts stay in-range
BAND2 = BROWS * GRID           # 2560 table rows
BSRC2 = BSRC * GRID            # 2640 source cols
QROWS = H // 4                 # 16 image rows per core
QPIX = QROWS * W               # 1024 pixels per core
NT = QPIX // 128               # 8 pixel tiles of 128 per core
E4 = 4 * C                     # table row length (elements)

# branch configs: (name, k, pad)
BRANCHES = [("b1", 1, 0), ("b2", 1, 0), ("b3", 3, 1), ("b4", 7, 3)]
HALO = 3                       # max pad over branches
HROWS = QROWS + 2 * HALO       # 22 halo rows
HCOLS = W + 2 * HALO           # 70 halo cols

_CACHE = {}


JGMAX = 13


def _jgroups(K):
    """Split kernel points into gather groups (<=JGMAX j per group)."""
    ngr = -(-K // JGMAX)
    base = K // ngr
    rem = K % ngr
    out, j = [], 0
    for g in range(ngr):
        n = base + (1 if g < rem else 0)
        out.append(list(range(j, j + n)))
        j += n
    return out


def build_program(no_collectives=False):
    nc = bacc.Bacc("TRN2", target_bir_lowering=False, num_devices=8)

    # ---------------- I/O declarations (per-core data supplied by host) ----
    inp = {}

    def di(name, shape, dtype):
        inp[name] = nc.dram_tensor(name, shape, dtype, kind="ExternalInput")
        return inp[name]

    di("pad80", [C, BSRC2], bf16)           # padded band (33 rows x 80), C-major
    di("x_halo", [C, HROWS * HCOLS], bf16)  # quarter + halo, zero padded
    for name, k, pad in BRANCHES:
        K = k * k
        CH = 3 * K
        di(f"{name}_wstk", [C, K, CH], bf16)      # [c, tap, out_ch]
        di(f"{name}_bias", [1, CH], bf16)         # offb|modb
        di(f"{name}_regw", [C, K, O], bf16)       # 2*regw, [c, j, o]
        di(f"{name}_by16", [128, NT, K], f32)     # base_y + 16 per (pix, t, j)
        di(f"{name}_bx16", [128, NT, K], f32)
        di(f"{name}_gamma", [1, O], f32)
        di(f"{name}_beta", [1, O], f32)
    di("pool_w", [C, O], f32)
    di("pool_gamma", [1, O], f32)
    di("pool_beta", [1, O], f32)
    di("fuse_wT", [C, 10, 128], bf16)        # chunk i: fuse_w[:, i*128+ch].T
    di("fuse_gamma", [1, 128], f32)
    di("fuse_beta", [1, 128], f32)

    out_d = nc.dram_tensor("out", [QPIX, 128], f32, kind="ExternalOutput")

    Ttbl = nc.dram_tensor("Ttbl", [BAND2, E4], bf16)

    with tile.TileContext(nc) as tc:
        with (
            tc.tile_pool(name="consts", bufs=1) as cst,
            tc.tile_pool(name="bigio", bufs=1) as bigio,
            tc.tile_pool(name="wts", bufs=1) as wts,
            tc.tile_pool(name="offs", bufs=1) as offs_p,
            tc.tile_pool(name="coef", bufs=1) as coef_p,
            tc.tile_pool(name="wrap", bufs=1) as wrap_p,
            tc.tile_pool(name="gat", bufs=2) as gat_p,
            tc.tile_pool(name="lrp", bufs=2) as lrp_p,
            tc.tile_pool(name="keep", bufs=1) as keep,
            tc.tile_pool(name="tmp", bufs=2) as tmp,
            tc.tile_pool(name="psA", bufs=2, space="PSUM") as psA,
            tc.tile_pool(name="psB", bufs=4, space="PSUM") as psB,
            tc.tile_pool(name="psS", bufs=1, space="PSUM") as psS,
            tc.tile_pool(name="dram", bufs=1, space="DRAM") as drm,
        ):
            # ---------------- constants -----------------------------------
            ones_col = cst.tile([128, 1], bf16, tag="ones_col")
            nc.vector.memset(ones_col[:], 1.0)
            ones_col_f = cst.tile([128, 1], f32, tag="ones_col_f")
            nc.vector.memset(ones_col_f[:], 1.0)
            ones_row = cst.tile([1, 128], bf16, tag="ones_row")
            nc.vector.memset(ones_row[:], 1.0)
            selcols = cst.tile([128, 8, 8], bf16, tag="selcols")
            nc.vector.memset(selcols[:], 0.0)
            for _s in range(8):
                nc.vector.memset(selcols[:, _s, _s : _s + 1], 1.0)
            stats_ps = psS.tile([8, O], f32, tag="statsps")

            # ---------------- pool branch --------------------------------
            xsum = keep.tile([128, 1], f32, tag="xsum")
            xmean = keep.tile([128, 1], f32, tag="xmean")
            # ---------------- 4-plane table build -------------------------
            p80 = bigio.tile([C, BSRC2], bf16, tag="pad80")
            nc.sync.dma_start(p80[:], inp["pad80"][:])
            nc.vector.tensor_reduce(
                out=xsum[:],
                in_=p80[:].rearrange("c (gy gx) -> c gy gx", gx=GRID)[
                    :, MARGIN : MARGIN + QROWS, MARGIN : MARGIN + W
                ],
                axis=mybir.AxisListType.XY,
                op=ALU.add,
            )
            ccx_in = drm.tile([128, 1], f32)
            ccx_out = drm.tile([128, 1], f32)
            nc.sync.dma_start(ccx_in[:], xsum[:])
            if no_collectives:
                nc.gpsimd.dma_start(ccx_out[:], ccx_in[:])
            else:
                nc.gpsimd.collective_compute(
                    "AllReduce",
                    ALU.add,
                    replica_groups=[[0, 1, 2, 3], [4, 5, 6, 7]],
                    ins=[ccx_in[:]],
                    outs=[ccx_out[:]],
                )
            nc.sync.dma_start(xsum[:], ccx_out[:])
            nc.vector.tensor_scalar_mul(xmean[:], xsum[:], 1.0 / HW)
            UBLK = BAND2 // 128  # 20
            Tv = Ttbl.ap().rearrange("(u a) e -> a u e", a=128)

            def emit_plane(pl_ap, p_i):
                pt = gat_p.tile([128, UBLK, 128], bf16, tag="gath")
                nc.sync.dma_start_transpose(pt[:], pl_ap)
                nc.sync.dma_start(Tv[:, :, 128 * p_i : 128 * (p_i + 1)], pt[:])

            # plane I directly from p80 (band rows 0..BROWS)
            emit_plane(p80[:, :BAND2], 0)
            # Dx over the full source (one col beyond band end stays in-range)
            dx = gat_p.tile([C, BSRC2], bf16, tag="gath")
            nc.vector.memset(dx[:, BSRC2 - 1 :], 0.0)
            nc.vector.tensor_tensor(
                out=dx[:, : BSRC2 - 1],
                in0=p80[:, 1:BSRC2],
                in1=p80[:, : BSRC2 - 1],
                op=ALU.subtract,
            )
            emit_plane(dx[:, :BAND2], 1)
            # Dxy (reads dx)
            dxy = gat_p.tile([C, BAND2], bf16, tag="gath")
            nc.vector.tensor_tensor(
                out=dxy[:],
                in0=dx[:, GRID : BAND2 + GRID],
                in1=dx[:, :BAND2],
                op=ALU.subtract,
            )
            emit_plane(dxy[:], 3)
            # Dy
            dy = gat_p.tile([C, BAND2], bf16, tag="gath")
            nc.vector.tensor_tensor(
                out=dy[:],
                in0=p80[:, GRID : BAND2 + GRID],
                in1=p80[:, :BAND2],
                op=ALU.subtract,
            )
            emit_plane(dy[:], 2)

            poolw_sb = wts.tile([C, O], f32, tag="poolw")
            nc.sync.dma_start(poolw_sb[:], inp["pool_w"][:])
            pp_ps = psS.tile([1, O], f32, tag="mini")
            nc.tensor.matmul(pp_ps[:], xmean[:], poolw_sb[:], start=True, stop=True)
            pp = keep.tile([1, O], f32, tag="poolrow")
            nc.vector.tensor_copy(pp[:], pp_ps[:])
            # GN over each group of 16 channels (spatial = 1)
            ps_sum = keep.tile([1, NGRP], f32, tag="pool_s")
            nc.vector.tensor_reduce(
                out=ps_sum[:],
                in_=pp[:].rearrange("a (g d) -> a g d", g=NGRP),
                axis=mybir.AxisListType.X,
                op=ALU.add,
            )
            pp_sq = keep.tile([1, O], f32, tag="gn_msb")
            nc.scalar.activation(out=pp_sq[:], in_=pp[:], func=ACTF.Square)
            ps_ssq = keep.tile([1, NGRP], f32, tag="pool_ss")
            nc.vector.tensor_reduce(
                out=ps_ssq[:],
                in_=pp_sq[:].rearrange("a (g d) -> a g d", g=NGRP),
                axis=mybir.AxisListType.X,
                op=ALU.add,
            )

            def gn_scale_bias(s_ap, ss_ap, count, gamma_ap, beta_ap, nch, ngrp, tagp):
                """Given group sums/sumsqs [1, ngrp], returns per-channel
                scale/bias rows [1, nch] (f32, partition 0)."""
                g = ngrp
                mean = keep.tile([1, NGRP], f32, tag="gn_mean")
                nc.vector.tensor_scalar_mul(mean[:, :g], s_ap, 1.0 / count)
                ex2 = keep.tile([1, NGRP], f32, tag="gn_ex2")
                nc.vector.tensor_scalar_mul(ex2[:, :g], ss_ap, 1.0 / count)
                msq = keep.tile([1, NGRP], f32, tag="gn_msq")
                nc.scalar.activation(out=msq[:, :g], in_=mean[:, :g], func=ACTF.Square)
                var = keep.tile([1, NGRP], f32, tag="gn_var")
                nc.vector.tensor_tensor(
                    out=var[:, :g], in0=ex2[:, :g], in1=msq[:, :g], op=ALU.subtract
                )
                nc.vector.tensor_scalar_add(var[:, :g], var[:, :g], float(EPS))
                std = keep.tile([1, NGRP], f32, tag="gn_std")
                nc.scalar.activation(out=std[:, :g], in_=var[:, :g], func=ACTF.Sqrt)
                rstd = keep.tile([1, NGRP], f32, tag="gn_rstd")
                nc.vector.reciprocal(rstd[:, :g], std[:, :g])
                d = nch // g
                scale_t = keep.tile([1, O], f32, tag="gn_scale")
                scale = scale_t[:, :nch]
                rstd_b = rstd[:, :g].rearrange("a (g o) -> a g o", o=1).to_broadcast([1, g, d])
                nc.vector.tensor_tensor(
                    out=scale.rearrange("a (g d) -> a g d", g=g),
                    in0=rstd_b,
                    in1=gamma_ap.rearrange("a (g d) -> a g d", g=g),
                    op=ALU.mult,
                )
                mean_b = mean[:, :g].rearrange("a (g o) -> a g o", o=1).to_broadcast([1, g, d])
                msb_t = keep.tile([1, O], f32, tag="gn_msb")
                msb = msb_t[:, :nch]
                nc.vector.tensor_tensor(
                    out=msb.rearrange("a (g d) -> a g d", g=g),
                    in0=mean_b,
                    in1=scale.rearrange("a (g d) -> a g d", g=g),
                    op=ALU.mult,
                )
                bias_t = keep.tile([1, O], f32, tag="gn_bias")
                bias = bias_t[:, :nch]
                nc.vector.tensor_tensor(
                    out=bias, in0=beta_ap, in1=msb, op=ALU.subtract
                )
                return scale, bias

            pgam = keep.tile([1, O], f32, tag="pool_gam")
            nc.sync.dma_start(pgam[:], inp["pool_gamma"][:])
            pbet = keep.tile([1, O], f32, tag="pool_bet")
            nc.sync.dma_start(pbet[:], inp["pool_beta"][:])
            psc, pbi = gn_scale_bias(
                ps_sum[:], ps_ssq[:], 16.0, pgam[:], pbet[:], O, NGRP, "pool"
            )
            x5row = keep.tile([1, O], bf16, tag="x5row")
            x5t = keep.tile([1, O], f32, tag="gn_msb")
            nc.vector.tensor_tensor(out=x5t[:], in0=pp[:], in1=psc, op=ALU.mult)
            nc.vector.tensor_tensor(out=x5t[:], in0=x5t[:], in1=pbi, op=ALU.add)
            nc.scalar.activation(out=x5row[:], in_=x5t[:], func=ACTF.Relu)

            # cat buffer: [128 pix, t, 10 chunks, 128 ch] bf16
            cat = keep.tile([128, 4, NT, 2, 128], bf16, tag="cat")
            x5cm2 = keep.tile([128, 2, 128], bf16, tag="x5cm2")
            # x5 C-major chunks via rank-1 matmul (x5col x ones_row)
            for hchunk in range(2):
                x5ps = psS.tile([128, 128], f32, tag="mini")
                nc.tensor.matmul(
                    x5ps[:],
                    x5row[0:1, 128 * hchunk : 128 * (hchunk + 1)],
                    ones_row[:],
                    start=True,
                    stop=True,
                )
                nc.vector.tensor_copy(x5cm2[:, hchunk, :], x5ps[:])
            # ---------------- branches ------------------------------------
            xh = bigio.tile([C, HROWS, HCOLS], bf16, tag="xhalo")
            nc.sync.dma_start(
                xh[:].rearrange("c r d -> c (r d)"), inp["x_halo"][:]
            )

            # collective staging
            cc1_in = drm.tile([1, 4 * 2 * NGRP], f32)
            cc1_out = drm.tile([1, 4 * 2 * NGRP], f32)
            cc2_in = drm.tile([1, 2 * NGRP], f32)
            cc2_out = drm.tile([1, 2 * NGRP], f32)

            out_br = []  # per-branch [128, NT, O] f32 (pre-GN)
            for bi, (name, k, pad) in enumerate(BRANCHES):
                K = k * k
                CH = 3 * K
                # ---- weights
                wstk = wts.tile([C, K, CH], bf16, tag="wstk")
                for _c0 in range(0, K, max(1, K // 4)):
                    _c1 = min(K, _c0 + max(1, K // 4))
                    nc.sync.dma_start(wstk[:, _c0:_c1, :], inp[f"{name}_wstk"][:, _c0:_c1, :])
                brow = wts.tile([1, CH], bf16, tag="brow")
                nc.sync.dma_start(brow[:], inp[f"{name}_bias"][:])
                regw = wts.tile([C, K, O], bf16, tag="regw")
                for _c0 in range(0, K, max(1, K // 4)):
                    _c1 = min(K, _c0 + max(1, K // 4))
                    nc.sync.dma_start(regw[:, _c0:_c1, :], inp[f"{name}_regw"][:, _c0:_c1, :])
                by16 = coef_p.tile([128, NT, K], f32, tag="by16")
                nc.sync.dma_start(by16[:], inp[f"{name}_by16"][:])
                bx16 = coef_p.tile([128, NT, K], f32, tag="bx16")
                nc.sync.dma_start(bx16[:], inp[f"{name}_bx16"][:])

                # ---- offset/mod convs: out^T [pix, CH] per tile
                offs = offs_p.tile([128, NT, CH], f32, tag="offs")
                for t in range(NT):
                    op_ps = psA.tile([128, 256], f32, tag="convps")
                    first = True
                    for ky in range(k):
                        for kx in range(k):
                            tap = ky * k + kx
                            r0 = 2 * t + ky - pad + HALO
                            c0 = kx - pad + HALO
                            for rr in range(2):
                                nc.tensor.matmul(
                                    op_ps[64 * rr : 64 * (rr + 1), :CH],
                                    xh[:, r0 + rr, c0 : c0 + W],
                                    wstk[:, tap, :],
                                    start=first,
                                    stop=False,
                                )
                            first = False
                    nc.tensor.matmul(
                        op_ps[:, :CH], ones_row[:], brow[:], start=False, stop=True
                    )
                    nc.vector.tensor_copy(offs[:, t, :], op_ps[:, :CH])

                # ---- mask + coefs + indices (b4: two tile-halves so the
                # first gathers can start before all offconv tiles finish)
                mask = coef_p.tile([128, NT, K], f32, tag="mask" if k == 7 else f"mask{name}")
                py = coef_p.tile([128, NT, K], f32, tag="py" if k == 7 else f"py{name}")
                px = coef_p.tile([128, NT, K], f32, tag="px" if k == 7 else f"px{name}")
                fy = coef_p.tile([128, NT, K], f32, tag="fy" if k == 7 else f"fy{name}")
                fx = coef_p.tile([128, NT, K], f32, tag="fx" if k == 7 else f"fx{name}")
                wy = coef_p.tile([128, NT, K], bf16, tag="wy" if k == 7 else f"wy{name}")
                wx = coef_p.tile([128, NT, K], bf16, tag="wx" if k == 7 else f"wx{name}")
                idx16 = coef_p.tile([128, NT, K], i16, tag="idx16" if k == 7 else f"idx16{name}")
                c1 = coef_p.tile([128, NT, K], f32, tag="c1" if k == 7 else f"c1{name}")
                c2 = coef_p.tile([128, NT, K], f32, tag="c2" if k == 7 else f"c2{name}")
                c3 = coef_p.tile([128, NT, K], f32, tag="c3" if k == 7 else f"c3{name}")
                WCOLS = NT * K * 8
                wrapt = wrap_p.tile([128, WCOLS], i16, tag="wrapt" if k == 7 else f"wrapt{name}")
                wv = wrapt[:].rearrange("p (t j r) -> p t j r", t=NT, j=K)
                halves = ((0, NT // 2), (NT // 2, NT)) if k == 7 else ((0, NT),)
                for h0, h1 in halves:
                    hs = slice(h0, h1)
                    nc.scalar.activation(
                        out=mask[:, hs], in_=offs[:, hs, 2 * K : 3 * K],
                        func=ACTF.Sigmoid,
                    )
                    nc.vector.tensor_tensor(
                        out=py[:, hs], in0=offs[:, hs, 0 : 2 * K : 2],
                        in1=by16[:, hs], op=ALU.add,
                    )
                    nc.vector.tensor_tensor(
                        out=px[:, hs], in0=offs[:, hs, 1 : 2 * K : 2],
                        in1=bx16[:, hs], op=ALU.add,
                    )
                    for crd, (pt_, ft_, wt_, himax) in {
                        "y": (py, fy, wy, float(BROWS - 2 + 16) + 0.99),
                        "x": (px, fx, wx, float(H + MARGIN - 2 + 16) + 0.99),
                    }.items():
                        # clamp into the (band-local for y) table grid
                        nc.vector.tensor_scalar_max(pt_[:, hs], pt_[:, hs], float(MARGIN))
                        nc.vector.tensor_scalar_min(pt_[:, hs], pt_[:, hs], himax)
                        # floor via int cast + correction (trunc/round agnostic)
                        icast = coef_p.tile([128, NT // (2 if k == 7 else 1), K], i32, tag="scr" if k == 7 else f"scr{name}")
                        corr = coef_p.tile([128, NT // (2 if k == 7 else 1), K], f32, tag="scr" if k == 7 else f"scr{name}")
                        nh = h1 - h0
                        nc.vector.tensor_copy(icast[:, :nh], pt_[:, hs])
                        nc.vector.tensor_copy(ft_[:, hs], icast[:, :nh])
                        nc.vector.tensor_tensor(
                            out=corr[:, :nh], in0=ft_[:, hs], in1=pt_[:, hs],
                            op=ALU.is_gt,
                        )
                        nc.vector.tensor_tensor(
                            out=ft_[:, hs], in0=ft_[:, hs], in1=corr[:, :nh],
                            op=ALU.subtract,
                        )
                        nc.vector.tensor_tensor(
                            out=wt_[:, hs], in0=pt_[:, hs], in1=ft_[:, hs],
                            op=ALU.subtract,
                        )
                    idxf = coef_p.tile([128, NT // (2 if k == 7 else 1), K], f32, tag="scr" if k == 7 else f"scr{name}")
                    nh = h1 - h0
                    nc.vector.scalar_tensor_tensor(
                        out=idxf[:, :nh], in0=fy[:, hs], scalar=float(GRID),
                        in1=fx[:, hs], op0=ALU.mult, op1=ALU.add,
                    )
                    nc.vector.tensor_scalar_add(
                        idxf[:, :nh], idxf[:, :nh],
                        -float((16 - MARGIN) * GRID + (16 - MARGIN)),
                    )
                    nc.vector.tensor_copy(idx16[:, hs], idxf[:, :nh])
                    # lerp coefs (x2 sigmoid scale folded into regw)
                    nc.vector.tensor_tensor(out=c1[:, hs], in0=mask[:, hs], in1=wx[:, hs], op=ALU.mult)
                    nc.vector.tensor_tensor(out=c2[:, hs], in0=mask[:, hs], in1=wy[:, hs], op=ALU.mult)
                    nc.vector.tensor_tensor(out=c3[:, hs], in0=c1[:, hs], in1=wy[:, hs], op=ALU.mult)
                    # index wrap for this half: cols are t-major so contiguous
                    for r in range(8):
                        nc.sync.dma_start(
                            wv[0:16, hs, :, r], idx16[16 * r : 16 * r + 16, hs, :]
                        )
                    for a in range(1, 8):
                        nc.sync.dma_start(
                            wrapt[16 * a : 16 * a + 16, h0 * K * 8 : h1 * K * 8],
                            wrapt[0:16, h0 * K * 8 : h1 * K * 8],
                        )

                jgs = _jgroups(K)
                sums = keep.tile([128, 2 * NGRP], f32, tag="stats")
                nc.vector.memset(sums[:], 0.0)
                obr = keep.tile([128, NT, O], bf16, tag=f"outbr{bi}")
                out_br.append(obr)
                for t in range(NT):
                    ps_out = psB.tile([128, O], f32, tag="ps_out")
                    for gi, jg in enumerate(jgs):
                        j0, nj = jg[0], len(jg)
                        gt = gat_p.tile([128, min(JGMAX, K), E4], bf16, tag="gath" if k >= 3 else f"gath{name}")
                        nidx = nj * 128
                        nc.gpsimd.dma_gather(
                            out_ap=gt[:, :nj, :],
                            in_ap=Ttbl.ap(),
                            idxs_ap=wrapt[:, (t * K + j0) * 8 : (t * K + j0 + nj) * 8],
                            num_idxs=nidx,
                            num_idxs_reg=nidx,
                            elem_size=E4,
                            single_packet=False,
                        )
                        st = lrp_p.tile([128, min(JGMAX, K), 128], bf16, tag="stile" if k >= 3 else f"stile{name}")
                        acc = lrp_p.tile([128, min(JGMAX, K), 128], bf16, tag="sacc" if k >= 3 else f"sacc{name}")
                        for jj in range(nj):
                            j = j0 + jj
                            msc = mask[:, t, j : j + 1]
                            c1s = c1[:, t, j : j + 1]
                            c2s = c2[:, t, j : j + 1]
                            c3s = c3[:, t, j : j + 1]
                            gI = gt[:, jj, 0:128]
                            gDx = gt[:, jj, 128:256]
                            gDy = gt[:, jj, 256:384]
                            gDxy = gt[:, jj, 384:512]
                            a_ = acc[:, jj, :]
                            if jj % 2 == 0:
                                nc.scalar.activation(
                                    out=a_, in_=gI, func=ACTF.Copy, scale=msc
                                )
                            else:
                                nc.vector.tensor_scalar_mul(a_, gI, msc)
                            nc.vector.scalar_tensor_tensor(
                                out=a_, in0=gDx, scalar=c1s, in1=a_,
                                op0=ALU.mult, op1=ALU.add,
                            )
                            nc.vector.scalar_tensor_tensor(
                                out=a_, in0=gDy, scalar=c2s, in1=a_,
                                op0=ALU.mult, op1=ALU.add,
                            )
                            nc.vector.scalar_tensor_tensor(
                                out=st[:, jj, :], in0=gDxy, scalar=c3s, in1=a_,
                                op0=ALU.mult, op1=ALU.add,
                            )
                        stT = lrp_p.tile([128, min(JGMAX, K), 128], bf16, tag="stileT" if k >= 3 else f"stileT{name}")
                        nc.sync.dma_start_transpose(
                            stT[:, :nj, :],
                            st[:, :nj, :].rearrange("p u e -> p (u e)"),
                        )
                        for jj in range(nj):
                            j = j0 + jj
                            nc.tensor.matmul(
                                ps_out[:],
                                stT[:, jj, :],
                                regw[:, j, :],
                                start=(j == 0),
                                stop=(j == K - 1),
                            )
                    # ---- epilogue: stash + PE one-hot stats accumulation
                    nc.scalar.activation(out=obr[:, t, :], in_=ps_out[:], func=ACTF.Copy)
                    sq = tmp.tile([128, O], bf16, tag="sqtile")
                    nc.scalar.activation(out=sq[:], in_=ps_out[:], func=ACTF.Square)
                    first_stats = (t == 0) and (bi == border[0])
                    nc.tensor.matmul(
                        stats_ps[:], selcols[:, bi, :], obr[:, t, :],
                        start=first_stats, stop=False, skip_group_check=True,
                    )
                    nc.tensor.matmul(
                        stats_ps[:], selcols[:, 4 + bi, :], sq[:],
                        start=False,
                        stop=(t == NT - 1) and (bi == border[-1]),
                        skip_group_check=True,
                    )
                # partition-reduce stats -> [2*NGRP, 1] -> DRAM staging
                sred = psS.tile([2 * NGRP, 1], f32, tag="mini")
                nc.tensor.matmul(sred[:], sums[:], ones_col_f[:], start=True, stop=True)
                sred_sb = tmp.tile([2 * NGRP, 1], f32, tag="sred_sb")
                nc.vector.tensor_copy(sred_sb[:], sred[:])
                nc.sync.dma_start(
                    cc1_in[0:1, bi * 2 * NGRP : (bi + 1) * 2 * NGRP].rearrange(
                        "a e -> e a"
                    ),
                    sred_sb[:],
                )

            # ---------------- AllReduce #1 --------------------------------
            if no_collectives:
                nc.gpsimd.dma_start(cc1_out[:], cc1_in[:])
            else:
                nc.gpsimd.collective_compute(
                    "AllReduce",
                    ALU.add,
                    replica_groups=[[0, 1, 2, 3], [4, 5, 6, 7]],
                    ins=[cc1_in[:]],
                    outs=[cc1_out[:]],
                )
            cc1_sb = keep.tile([1, 4 * 2 * NGRP], f32, tag="cc1_sb")
            nc.sync.dma_start(cc1_sb[:], cc1_out[:])

            # ---------------- GN apply per branch + cat -------------------
            for bi, (name, k, pad) in enumerate(BRANCHES):
                gam = keep.tile([1, O], f32, tag="gamld")
                nc.sync.dma_start(gam[:], inp[f"{name}_gamma"][:])
                bet = keep.tile([1, O], f32, tag="betld")
                nc.sync.dma_start(bet[:], inp[f"{name}_beta"][:])
                s_ap = cc1_sb[0:1, bi * NGRP : (bi + 1) * NGRP]
                ss_ap = cc1_sb[0:1, 64 + bi * NGRP : 64 + (bi + 1) * NGRP]
                scale, bias = gn_scale_bias(
                    s_ap, ss_ap, float(16 * HW), gam[:], bet[:], O, NGRP, "br"
                )
                scale_b16 = keep.tile([1, O], bf16, tag="scale_b16")
                nc.vector.tensor_copy(scale_b16[:], scale)
                bias_b16 = keep.tile([1, O], bf16, tag="bias_b16")
                nc.vector.tensor_copy(bias_b16[:], bias)
                scb = keep.tile([128, O], bf16, tag="scb")
                nc.gpsimd.partition_broadcast(scb[:], scale_b16[:])
                bib = keep.tile([128, O], bf16, tag="bib")
                nc.gpsimd.partition_broadcast(bib[:], bias_b16[:])
                obr = out_br[bi]
                for t in range(NT):
                    nrm = tmp.tile([128, O], bf16, tag="nrm")
                    nc.vector.tensor_tensor(
                        out=nrm[:], in0=obr[:, t, :], in1=scb[:], op=ALU.mult
                    )
                    nc.vector.tensor_tensor(
                        out=nrm[:], in0=nrm[:], in1=bib[:], op=ALU.add
                    )
                    nc.scalar.activation(out=obr[:, t, :], in_=nrm[:], func=ACTF.Relu)
                nc.sync.dma_start_transpose(
                    cat[:, bi, :, :, :].rearrange("p t h e -> p (t h) e"),
                    obr[:].rearrange("p t c -> p (t c)"),
                )

            # ---------------- fuse conv + GN ------------------------------
            fw = wts.tile([C, 10, 128], bf16, tag="fusew")
            nc.sync.dma_start(fw[:], inp["fuse_wT"][:])
            fsums = keep.tile([128, 2 * NGRP], f32, tag="fsums")
            nc.vector.memset(fsums[:], 0.0)
            fout = keep.tile([128, NT, 128], bf16, tag="fout")
            for t in range(NT):
                fps = psB.tile([128, O], f32, tag="ps_out")
                for i in range(10):
                    lhsT = (
                        cat[:, i // 2, t, i % 2, :] if i < 8 else x5cm2[:, i - 8, :]
                    )
                    nc.tensor.matmul(
                        fps[:, :128],
                        lhsT,
                        fw[:, i, :],
                        start=(i == 0),
                        stop=(i == 9),
                    )
                nc.scalar.activation(out=fout[:, t, :], in_=fps[:, :128], func=ACTF.Copy)
                fsq = tmp.tile([128, 128], f32, tag="fsq")
                nc.scalar.activation(out=fsq[:], in_=fps[:, :128], func=ACTF.Square)
                fpart = tmp.tile([128, 2 * NGRP], f32, tag="fpart")
                nc.vector.tensor_reduce(
                    out=fpart[:, :NGRP],
                    in_=fps[:, :128].rearrange("p (g d) -> p g d", g=NGRP),
                    axis=mybir.AxisListType.X,
                    op=ALU.add,
                )
                nc.vector.tensor_reduce(
                    out=fpart[:, NGRP:],
                    in_=fsq[:].rearrange("p (g d) -> p g d", g=NGRP),
                    axis=mybir.AxisListType.X,
                    op=ALU.add,
                )
                nc.vector.tensor_tensor(
                    out=fsums[:], in0=fsums[:], in1=fpart[:], op=ALU.add
                )
            fred = psS.tile([2 * NGRP, 1], f32, tag="mini")
            nc.tensor.matmul(fred[:], fsums[:], ones_col_f[:], start=True, stop=True)
            fred_sb = tmp.tile([2 * NGRP, 1], f32, tag="sred_sb")
            nc.vector.tensor_copy(fred_sb[:], fred[:])
            nc.sync.dma_start(cc2_in[:].rearrange("a e -> e a"), fred_sb[:])
            if no_collectives:
                nc.gpsimd.dma_start(cc2_out[:], cc2_in[:])
            else:
                nc.gpsimd.collective_compute(
                    "AllReduce",
                    ALU.add,
                    replica_groups=[[0, 1, 2, 3], [4, 5, 6, 7]],
                    ins=[cc2_in[:]],
                    outs=[cc2_out[:]],
                )
            cc2_sb = keep.tile([1, 2 * NGRP], f32, tag="cc2_sb")
            nc.sync.dma_start(cc2_sb[:], cc2_out[:])
            fgam_t = keep.tile([1, O], f32, tag="gamld")
            fgam = fgam_t[:, :128]
            nc.sync.dma_start(fgam, inp["fuse_gamma"][:])
            fbet_t = keep.tile([1, O], f32, tag="betld")
            fbet = fbet_t[:, :128]
            nc.sync.dma_start(fbet, inp["fuse_beta"][:])
            fscale, fbias = gn_scale_bias(
                cc2_sb[0:1, :NGRP],
                cc2_sb[0:1, NGRP:],
                float(8 * HW),
                fgam,
                fbet,
                128,
                NGRP,
                "fuse",
            )
            fscb_t = keep.tile([128, O], f32, tag="scb")
            fscb = fscb_t[:, :128]
            nc.gpsimd.partition_broadcast(fscb, fscale)
            fbib_t = keep.tile([128, O], f32, tag="bib")
            fbib = fbib_t[:, :128]
            nc.gpsimd.partition_broadcast(fbib, fbias)
            for t in range(NT):
                fn = tmp.tile([128, 128], f32, tag="fn")
                nc.vector.tensor_tensor(
                    out=fn[:], in0=fout[:, t, :], in1=fscb, op=ALU.mult
                )
                nc.vector.tensor_tensor(out=fn[:], in0=fn[:], in1=fbib, op=ALU.add)
                fr = tmp.tile([128, 128], f32, tag="fr")
                nc.scalar.activation(out=fr[:], in_=fn[:], func=ACTF.Relu)
                nc.sync.dma_start(out_d[128 * t : 128 * (t + 1), :], fr[:])

    nc.compile()
    return nc


# ======================= host-side data prep ==============================

def _prep_core_inputs(inputs, core):
    b, q = core // 4, core % 4
    x = np.asarray(inputs["x"])[b]            # [C, H, W] f32
    xc = x.reshape(C, HW).astype(np.float32)
    m = {}
    pad = np.zeros((C, H + 2 * MARGIN + 1, GRID), np.float32)
    pad[:, MARGIN : MARGIN + H, MARGIN : MARGIN + W] = x
    r0 = q * QROWS  # band starts at image row q*QROWS - MARGIN = pad row q*QROWS
    m["pad80"] = pad[:, r0 : r0 + BSRC, :].reshape(C, BSRC2).astype(
        ml_dtypes.bfloat16
    )
    halo = np.zeros((C, HROWS, HCOLS), np.float32)
    r0 = q * QROWS - HALO
    for rr in range(HROWS):
        rimg = r0 + rr
        if 0 <= rimg < H:
            halo[:, rr, HALO : HALO + W] = x[:, rimg, :]
    m["x_halo"] = halo.reshape(C, HROWS * HCOLS).astype(ml_dtypes.bfloat16)

    yq = np.arange(QPIX) // W + q * QROWS     # image row per pixel
    xq = np.arange(QPIX) % W
    # pixel i at (partition i%128, tile i//128)
    yq = yq.reshape(NT, 128).T                # [128, NT]
    xq = xq.reshape(NT, 128).T

    for name, k, pad_ in BRANCHES:
        K = k * k
        offw = np.asarray(inputs[f"{name}_offw"], np.float32)
        offb = np.asarray(inputs[f"{name}_offb"], np.float32)
        modw = np.asarray(inputs[f"{name}_modw"], np.float32)
        modb = np.asarray(inputs[f"{name}_modb"], np.float32)
        regw = np.asarray(inputs[f"{name}_regw"], np.float32)
        CH = 3 * K
        wstk = np.zeros((C, K, CH), np.float32)
        for ky in range(k):
            for kx in range(k):
                tap = ky * k + kx
                wstk[:, tap, : 2 * K] = offw[:, :, ky, kx].T
                wstk[:, tap, 2 * K :] = modw[:, :, ky, kx].T
        m[f"{name}_wstk"] = wstk.astype(ml_dtypes.bfloat16)
        m[f"{name}_bias"] = np.concatenate([offb, modb])[None, :].astype(
            ml_dtypes.bfloat16
        )
        m[f"{name}_regw"] = (
            (2.0 * regw.reshape(O, C, K)).transpose(1, 2, 0).astype(ml_dtypes.bfloat16)
        )
        ky, kx = np.meshgrid(np.arange(k), np.arange(k), indexing="ij")
        ky = ky.reshape(-1).astype(np.float32)
        kx = kx.reshape(-1).astype(np.float32)
        m[f"{name}_by16"] = (
            yq[:, :, None] - pad_ + ky[None, None, :] + 16.0 - q * QROWS
        ).astype(np.float32)
        m[f"{name}_bx16"] = (
            xq[:, :, None] - pad_ + kx[None, None, :] + 16.0
        ).astype(np.float32)
        m[f"{name}_gamma"] = np.asarray(inputs[f"{name}_gng"], np.float32)[None, :]
        m[f"{name}_beta"] = np.asarray(inputs[f"{name}_gnb"], np.float32)[None, :]

    m["pool_w"] = np.asarray(inputs["pool_w"], np.float32).reshape(O, C).T.copy()
    m["pool_gamma"] = np.asarray(inputs["pool_gng"], np.float32)[None, :]
    m["pool_beta"] = np.asarray(inputs["pool_gnb"], np.float32)[None, :]
    fw = np.asarray(inputs["fuse_w"], np.float32).reshape(128, 1280)
    fwT = np.zeros((C, 10, 128), np.float32)
    for i in range(10):
        fwT[:, i, :] = fw[:, i * 128 : (i + 1) * 128].T
    m["fuse_wT"] = fwT.astype(ml_dtypes.bfloat16)
    m["fuse_gamma"] = np.asarray(inputs["fuse_gng"], np.float32)[None, :]
    m["fuse_beta"] = np.asarray(inputs["fuse_gnb"], np.float32)[None, :]
    return m


def kernel(**inputs):
    if "nc" not in _CACHE:
        _CACHE["nc"] = build_program()
    nc = _CACHE["nc"]
    in_maps = [_prep_core_inputs(inputs, core) for core in range(8)]
    res = run_bass_kernel_spmd(nc, in_maps, list(range(8)))
    _CACHE["last_results"] = res
    out = np.zeros((B, 128, H, W), np.float32)
    for core in range(8):
        b, q = core // 4, core % 4
        oc = res.results[core]["out"]          # [QPIX, 128]
        # pixel i at (partition i%128, row-tile i//128): out rows are pixel-major
        oc = oc.reshape(QROWS, W, 128).transpose(2, 0, 1)
        out[b, :, q * QROWS : (q + 1) * QROWS, :] = oc
    return out
